# revision 11
# baseline (speedup 1.0000x reference)
"""Trainium2 Bass kernel for CLSProcess: diagonal linear recurrence
state_t = y_t * state_{t-1} + x_t * z_t over [B=8, T=4096, units=1024].

Sharding: batch across the 8 cores (one batch element per core).

v6 design — NO serial inter-block chain on device. The block recurrence
s_k = A_k s_{k-1} + b_k is solved with host-precomputed weights:

  - Host (f64, exact): per-block decay matrices M'_k[s,t] = x_s*prod(y)
    (main-matmul lhsT), the triangular inter-block propagator
    W[r,j] = prod_{i=j+1..r} A_i, "stacked matvec" weights
    mv2_j = outer(mlast_j, W[:,j]) and carry selectors
    selt_k[s,t] = I[s==k-1] * p_{k,t}. All shipped as bf16 sidecars.
  - Device (per core, single pass, N=1024 bf16 matmuls into 2-bank
    PSUM tiles):
      main:   po_k  = M'_k.T @ z_k           (start, no stop)
      matvec: ps_S += mv2_k.T @ z_k          (stacked: row r of ps_S
              accumulates W[r,k]*b_k, so S rows <= k are FINAL after
              matvec k — the inter-block scan happens inside PSUM
              accumulation, progressively, with no barrier)
      every 2 blocks: copy ps_S -> Stile (bf16, triple-buffered; two
              [32,512] copies split across scalar+vector)
      carry:  po_k += selt_k.T @ Stile       (K=32, start=False)
      drain:  one [128,1024] CAST -> bf16 out tile -> DMA out
  - Carries/drains are emitted DELAY=2 blocks behind mains; with CH=2
    the S-copy for row k-1 lands >=2 iterations before carry_k needs
    it, so the in-order PE queue never stalls on the S pipeline.
  - PSUM: 3 po tiles (2 banks each) + ps_S (2 banks) = 8 banks.
  - z is loaded via per-chunk SBUF tiles (sized small-first so block 0
    starts ~1us after the DMA preamble); M' sidecar is split in 4
    separate tiles so main_0 doesn't wait on the full 1MB transfer.
    I/O bf16; output written as [ND, 128, 2048] (2-block 4KB lines).
"""

import numpy as np
import ml_dtypes

import concourse.bacc as bacc
import concourse.bass as bass
import concourse.mybir as mybir
import concourse.tile as tile
from concourse.bass_utils import run_bass_kernel_spmd

B = 8
T = 4096
F = 1026
U = 1024
L = 128
NB = T // L  # 32 blocks
ND = NB // 2  # 16 two-block DMA groups
DELAY = 2  # carry/drain emission lag behind mains (blocks)
# z DMA chunking (in blocks): small chunks first so the pipeline
# starts early, large later to cut DMA instruction count
ZCHUNKS = (1, 1, 2, 2, 2, 4, 6, 7, 7)
NWARM = 6  # HAM warm-up dummy matmuls during the initial load phase
f32 = mybir.dt.float32
bf16 = mybir.dt.bfloat16
nbf16 = ml_dtypes.bfloat16


def build_nc() -> bass.Bass:
    nc = bacc.Bacc()
    # zin[p, k, c] = z_{k*L + p, c}  (per-p 64KB contiguous)
    zin = nc.dram_tensor("zin", [L, NB, U], bf16, kind="ExternalInput")
    # mtT[s, k*L + t] = M'_k[t, s] = x_{kL+s} * prod_{r=kL+s+1..kL+t} y_r
    mtT = nc.dram_tensor("mtT", [L, NB * L], bf16, kind="ExternalInput")
    # mvT[s, k*NB + r] = W[r, k] * M'_k[L-1, s]
    mvT = nc.dram_tensor("mvT", [L, NB * NB], bf16, kind="ExternalInput")
    # seltT[s, k*L + t] = I[s == k-1] * prod_{r=kL..kL+t} y_r
    seltT = nc.dram_tensor("seltT", [NB, NB * L], bf16, kind="ExternalInput")
    # out[d, p, j*U + c] = out_{(2d+j)*L + p, c}
    out = nc.dram_tensor("out", [ND, L, 2 * U], bf16, kind="ExternalOutput")

    with tile.TileContext(nc) as tc:
        with (
            tc.tile_pool(name="const", bufs=1) as constp,
            tc.tile_pool(name="zpool", bufs=1) as zp,
            tc.tile_pool(name="stilep", bufs=3) as stp,
            tc.tile_pool(name="otpool", bufs=4) as otp,
            tc.tile_pool(name="po", bufs=3, space="PSUM") as pop,
            tc.tile_pool(name="psS", bufs=1, space="PSUM") as psp,
        ):
            # mt split so early blocks only wait on their own slice
            MTCH = (2, 6, 8, 8, 8)
            mts = []  # (tile, k0, nblocks)
            k0 = 0
            for qi, qw in enumerate(MTCH):
                t = constp.tile([L, qw * L], bf16, tag=f"mt{qi}", name=f"mt{qi}")
                mts.append((t, k0, qw))
                k0 += qw
            MVH = 4  # mv head blocks
            mvh = constp.tile([L, MVH * NB], bf16, tag="mvh")
            mvt = constp.tile([L, (NB - MVH) * NB], bf16, tag="mvt")
            SEH = 8  # selt head blocks
            seh = constp.tile([NB, SEH * L], bf16, tag="seh")
            set_ = constp.tile([NB, (NB - SEH) * L], bf16, tag="set")
            zts = []  # (tile, k0, nblocks)
            zmap = {}  # block k -> (tile, block offset in chunk)
            k0 = 0
            for ci, cw in enumerate(ZCHUNKS):
                zt = zp.tile([L, cw * U], bf16, tag=f"tz{ci}", name=f"tz{ci}")
                zts.append((zt, k0, cw))
                for kk in range(cw):
                    zmap[k0 + kk] = (zt, kk)
                k0 += cw

            # warm-up tile for HAM dummy matmuls (no DMA dependency)
            warm = constp.tile([L, 640], bf16, tag="warm")
            nc.vector.memset(warm[:], 0.0)

            # DMA emission order: first-needed first
            def zdma(i):
                zt, zk0, zw = zts[i]
                nc.sync.dma_start(zt[:], zin[:, zk0 : zk0 + zw, :])

            def mtdma(i):
                t, tk0, tw = mts[i]
                nc.sync.dma_start(t[:], mtT[:, tk0 * L : (tk0 + tw) * L])

            zdma(0)
            mtdma(0)
            nc.sync.dma_start(mvh[:], mvT[:, 0 : MVH * NB])
            zdma(1)
            nc.sync.dma_start(seh[:], seltT[:, 0 : SEH * L])
            mtdma(1)
            zdma(2)
            zdma(3)
            mtdma(2)
            nc.sync.dma_start(mvt[:], mvT[:, MVH * NB :])
            zdma(4)
            nc.sync.dma_start(set_[:], seltT[:, SEH * L :])
            mtdma(3)
            zdma(5)
            mtdma(4)
            zdma(6)
            zdma(7)
            zdma(8)

            def rhs_z(k, h):
                zt, kk = zmap[k]
                off = kk * U + h * 512
                return zt[:, off : off + 512]

            def lhs_mt(k):
                for t, tk0, tw in mts:
                    if tk0 <= k < tk0 + tw:
                        return t[:, (k - tk0) * L : (k - tk0 + 1) * L]
                raise AssertionError(k)

            def lhs_mv(k):
                if k < MVH:
                    return mvh[:, k * NB : (k + 1) * NB]
                return mvt[:, (k - MVH) * NB : (k - MVH + 1) * NB]

            def lhs_selt(k):
                if k < SEH:
                    return seh[0:NB, k * L : (k + 1) * L]
                return set_[0:NB, (k - SEH) * L : (k - SEH + 1) * L]

            psS = (
                psp.tile([NB, 512], f32, tag="psSa", name="psSa"),
                psp.tile([NB, 512], f32, tag="psSb", name="psSb"),
            )
            stiles = {}
            pos = {}
            ots = {}

            def emit_front(k):
                po_a = pop.tile([L, 512], f32, tag="poa")
                po_b = pop.tile([L, 512], f32, tag="pob")
                po = (po_a, po_b)
                for h in range(2):
                    nc.tensor.matmul(
                        po[h][:, :],
                        lhs_mt(k),
                        rhs_z(k, h),
                        start=True,
                        stop=(k == 0),
                    )
                pos[k] = po
                if k < NB - 1:
                    for h in range(2):
                        nc.tensor.matmul(
                            psS[h][:, :],
                            lhs_mv(k),
                            rhs_z(k, h),
                            start=(k == 0),
                            stop=(k == NB - 2),
                        )
                if k % 2 == 1:
                    c = k // 2
                    st = stp.tile([NB, U], bf16, tag="st")
                    nc.scalar.copy(st[:, 0:512], psS[0][0:NB, :])
                    nc.vector.tensor_copy(st[:, 512:U], psS[1][0:NB, :])
                    stiles[c] = st

            def emit_back(k):
                po = pos.pop(k)
                if k > 0:
                    # S row k-1 is covered by the copy after matvec of
                    # block b = k-1 if odd else k (chunk index b//2)
                    b = k - 1 if (k - 1) % 2 == 1 else k
                    st = stiles[b // 2]
                    for h in range(2):
                        nc.tensor.matmul(
                            po[h][:, :],
                            lhs_selt(k),
                            st[0:NB, h * 512 : h * 512 + 512],
                            start=False,
                            stop=True,
                        )
                if k % 2 == 0:
                    ot = otp.tile([L, 2 * U], bf16, tag="ot")
                    ots[k // 2] = ot
                ot = ots[k // 2]
                o0 = (k % 2) * U
                nc.vector.tensor_copy(ot[:, o0 : o0 + 512], po[0][:, :])
                nc.scalar.copy(ot[:, o0 + 512 : o0 + U], po[1][:, :])
                if k % 2 == 1:
                    d = k // 2
                    ot_full = ots.pop(d)
                    if d >= ND - 2:
                        nc.sync.dma_start(out[d, :, 0:U], ot_full[:, 0:U])
                        nc.sync.dma_start(out[d, :, U:], ot_full[:, U:])
                    else:
                        nc.sync.dma_start(out[d, :, :], ot_full[:])

            for w in range(NWARM):
                pw = pop.tile([L, 512], f32, tag="poa" if w % 2 == 0 else "pob")
                nc.tensor.matmul(
                    pw[:, :], warm[:, 0:L], warm[:, L : L + 512],
                    start=True, stop=True,
                )
            for k in range(NB):
                if k >= DELAY:
                    emit_back(k - DELAY)
                emit_front(k)
            for k in range(NB - DELAY, NB):
                emit_back(k)
    nc.finalize()
    return nc


_NC = None


def _get_nc() -> bass.Bass:
    global _NC
    if _NC is None:
        _NC = build_nc()
    return _NC


def prep_in_maps(x: np.ndarray) -> list[dict]:
    maps = []
    sidx = np.arange(L)
    smask = sidx[None, :, None] <= sidx[None, None, :]
    for c in range(B):
        xs = x[c, :, 0].astype(np.float64)
        ys = x[c, :, 1].astype(np.float64)
        z = x[c, :, 2:]
        yb = ys.reshape(NB, L)
        xb = xs.reshape(NB, L)
        cp = np.cumprod(yb, axis=1)  # cp[k,t] = prod_{r=0..t} y_{kL+r}
        ratio = cp[:, None, :] / cp[:, :, None]  # prod_{s+1..t}
        mt = xb[:, :, None] * ratio * smask  # [k, s, t]
        mlast = mt[:, :, L - 1]  # [k, s]
        A = cp[:, L - 1]
        W = np.zeros((NB, NB))
        for r in range(NB):
            W[r, r] = 1.0
            if r:
                W[r, :r] = W[r - 1, :r] * A[r]
        mv2 = mlast[:, :, None] * W.T[:, None, :]  # [k, s, r]
        selt = np.zeros((NB, NB, L))  # [s, k, t]
        for k in range(1, NB):
            selt[k - 1, k, :] = cp[k]

        # zin[p, k, c] = z[k*L + p, c]
        zb = (
            np.ascontiguousarray(z)
            .astype(nbf16)
            .reshape(NB, L, U)
            .transpose(1, 0, 2)
        )
        maps.append(
            {
                "zin": np.ascontiguousarray(zb),
                "mtT": np.ascontiguousarray(
                    mt.transpose(1, 0, 2).reshape(L, NB * L).astype(nbf16)
                ),
                "mvT": np.ascontiguousarray(
                    mv2.transpose(1, 0, 2).reshape(L, NB * NB).astype(nbf16)
                ),
                "seltT": np.ascontiguousarray(
                    selt.reshape(NB, NB * L).astype(nbf16)
                ),
            }
        )
    return maps


def unpack_out(outb: np.ndarray) -> np.ndarray:
    # outb [B, ND, L, 2U]: out[d, p, j*U+c] = res[(2d+j)L+p, c]
    o = outb.reshape(B, ND, L, 2, U)
    o = o.transpose(0, 1, 3, 2, 4)  # [B, d, j, p, c]
    return np.ascontiguousarray(o).reshape(B, T, U).astype(np.float32)


def kernel(**inputs: np.ndarray) -> np.ndarray:
    x = np.ascontiguousarray(inputs["inputs"], dtype=np.float32)
    assert x.shape == (B, T, F), x.shape
    nc = _get_nc()
    res = run_bass_kernel_spmd(nc, prep_in_maps(x), core_ids=list(range(B)))
    outb = np.stack([res.results[c]["out"] for c in range(B)], axis=0)
    return unpack_out(outb)


# revision 12
# speedup vs baseline: 1.0498x; 1.0498x over previous
"""Trainium2 Bass kernel for CLSProcess: diagonal linear recurrence
state_t = y_t * state_{t-1} + x_t * z_t over [B=8, T=4096, units=1024].

Sharding: batch across the 8 cores (one batch element per core).

v6 design — NO serial inter-block chain on device. The block recurrence
s_k = A_k s_{k-1} + b_k is solved with host-precomputed weights:

  - Host (f64, exact): per-block decay matrices M'_k[s,t] = x_s*prod(y)
    (main-matmul lhsT), the triangular inter-block propagator
    W[r,j] = prod_{i=j+1..r} A_i, "stacked matvec" weights
    mv2_j = outer(mlast_j, W[:,j]) and carry selectors
    selt_k[s,t] = I[s==k-1] * p_{k,t}. All shipped as bf16 sidecars.
  - Device (per core, single pass, N=1024 bf16 matmuls into 2-bank
    PSUM tiles):
      main:   po_k  = M'_k.T @ z_k           (start, no stop)
      matvec: ps_S += mv2_k.T @ z_k          (stacked: row r of ps_S
              accumulates W[r,k]*b_k, so S rows <= k are FINAL after
              matvec k — the inter-block scan happens inside PSUM
              accumulation, progressively, with no barrier)
      every 2 blocks: copy ps_S -> Stile (bf16, triple-buffered; two
              [32,512] copies split across scalar+vector)
      carry:  po_k += selt_k.T @ Stile       (K=32, start=False)
      drain:  one [128,1024] CAST -> bf16 out tile -> DMA out
  - Carries/drains are emitted DELAY=2 blocks behind mains; with CH=2
    the S-copy for row k-1 lands >=2 iterations before carry_k needs
    it, so the in-order PE queue never stalls on the S pipeline.
  - PSUM: 3 po tiles (2 banks each) + ps_S (2 banks) = 8 banks.
  - z is loaded via per-chunk SBUF tiles (sized small-first so block 0
    starts ~1us after the DMA preamble); M' sidecar is split in 4
    separate tiles so main_0 doesn't wait on the full 1MB transfer.
    I/O bf16; output written as [ND, 128, 2048] (2-block 4KB lines).
"""

import numpy as np
import ml_dtypes

import concourse.bacc as bacc
import concourse.bass as bass
import concourse.mybir as mybir
import concourse.tile as tile
from concourse.bass_utils import run_bass_kernel_spmd

B = 8
T = 4096
F = 1026
U = 1024
L = 128
NB = T // L  # 32 blocks
ND = NB // 2  # 16 two-block DMA groups
DELAY = 2  # carry/drain emission lag behind mains (blocks)
# z DMA chunking (in blocks): small chunks first so the pipeline
# starts early, large later to cut DMA instruction count
ZCHUNKS = (1, 1, 2, 2, 2, 4, 6, 7, 7)
NWARM = 6  # HAM warm-up dummy matmuls during the initial load phase
f32 = mybir.dt.float32
bf16 = mybir.dt.bfloat16
nbf16 = ml_dtypes.bfloat16


def build_nc() -> bass.Bass:
    nc = bacc.Bacc()
    # zin[p, k, c] = z_{k*L + p, c}  (per-p 64KB contiguous)
    zin = nc.dram_tensor("zin", [L, NB, U], bf16, kind="ExternalInput")
    # mtT[s, k*L + t] = M'_k[t, s] = x_{kL+s} * prod_{r=kL+s+1..kL+t} y_r
    mtT = nc.dram_tensor("mtT", [L, NB * L], bf16, kind="ExternalInput")
    # mvT[s, k*NB + r] = W[r, k] * M'_k[L-1, s]
    mvT = nc.dram_tensor("mvT", [L, NB * NB], bf16, kind="ExternalInput")
    # seltT[s, k*L + t] = I[s == k-1] * prod_{r=kL..kL+t} y_r
    seltT = nc.dram_tensor("seltT", [NB, NB * L], bf16, kind="ExternalInput")
    # out[d, p, j*U + c] = out_{(2d+j)*L + p, c}
    out = nc.dram_tensor("out", [ND, L, 2 * U], bf16, kind="ExternalOutput")

    with tile.TileContext(nc) as tc:
        with (
            tc.tile_pool(name="const", bufs=1) as constp,
            tc.tile_pool(name="zpool", bufs=1) as zp,
            tc.tile_pool(name="stilep", bufs=3) as stp,
            tc.tile_pool(name="otpool", bufs=4) as otp,
            tc.tile_pool(name="po", bufs=3, space="PSUM") as pop,
            tc.tile_pool(name="psS", bufs=1, space="PSUM") as psp,
        ):
            # mt split so early blocks only wait on their own slice
            MTCH = (2, 6, 8, 8, 8)
            mts = []  # (tile, k0, nblocks)
            k0 = 0
            for qi, qw in enumerate(MTCH):
                t = constp.tile([L, qw * L], bf16, tag=f"mt{qi}", name=f"mt{qi}")
                mts.append((t, k0, qw))
                k0 += qw
            MVH = 4  # mv head blocks
            mvh = constp.tile([L, MVH * NB], bf16, tag="mvh")
            mvt = constp.tile([L, (NB - MVH) * NB], bf16, tag="mvt")
            SEH = 8  # selt head blocks
            seh = constp.tile([NB, SEH * L], bf16, tag="seh")
            set_ = constp.tile([NB, (NB - SEH) * L], bf16, tag="set")
            zts = []  # (tile, k0, nblocks)
            zmap = {}  # block k -> (tile, block offset in chunk)
            k0 = 0
            for ci, cw in enumerate(ZCHUNKS):
                zt = zp.tile([L, cw * U], bf16, tag=f"tz{ci}", name=f"tz{ci}")
                zts.append((zt, k0, cw))
                for kk in range(cw):
                    zmap[k0 + kk] = (zt, kk)
                k0 += cw

            # warm-up tile for HAM dummy matmuls (no DMA dependency)
            warm = constp.tile([L, 640], bf16, tag="warm")
            nc.vector.memset(warm[:], 0.0)

            # DMA emission order: first-needed first
            def zdma(i):
                zt, zk0, zw = zts[i]
                nc.sync.dma_start(zt[:], zin[:, zk0 : zk0 + zw, :])

            def mtdma(i):
                t, tk0, tw = mts[i]
                nc.sync.dma_start(t[:], mtT[:, tk0 * L : (tk0 + tw) * L])

            zdma(0)
            mtdma(0)
            nc.sync.dma_start(mvh[:], mvT[:, 0 : MVH * NB])
            zdma(1)
            nc.sync.dma_start(seh[:], seltT[:, 0 : SEH * L])
            mtdma(1)
            zdma(2)
            zdma(3)
            mtdma(2)
            nc.sync.dma_start(mvt[:], mvT[:, MVH * NB :])
            zdma(4)
            nc.sync.dma_start(set_[:], seltT[:, SEH * L :])
            mtdma(3)
            zdma(5)
            mtdma(4)
            zdma(6)
            zdma(7)
            zdma(8)

            def rhs_z(k, h):
                zt, kk = zmap[k]
                off = kk * U + h * 512
                return zt[:, off : off + 512]

            def lhs_mt(k):
                for t, tk0, tw in mts:
                    if tk0 <= k < tk0 + tw:
                        return t[:, (k - tk0) * L : (k - tk0 + 1) * L]
                raise AssertionError(k)

            def lhs_mv(k):
                if k < MVH:
                    return mvh[:, k * NB : (k + 1) * NB]
                return mvt[:, (k - MVH) * NB : (k - MVH + 1) * NB]

            def lhs_selt(k):
                if k < SEH:
                    return seh[0:NB, k * L : (k + 1) * L]
                return set_[0:NB, (k - SEH) * L : (k - SEH + 1) * L]

            psS = (
                psp.tile([NB, 512], f32, tag="psSa", name="psSa"),
                psp.tile([NB, 512], f32, tag="psSb", name="psSb"),
            )
            stiles = {}
            pos = {}
            ots = {}

            def emit_front(k):
                po_a = pop.tile([L, 512], f32, tag="poa")
                po_b = pop.tile([L, 512], f32, tag="pob")
                po = (po_a, po_b)
                for h in range(2):
                    nc.tensor.matmul(
                        po[h][:, :],
                        lhs_mt(k),
                        rhs_z(k, h),
                        start=True,
                        stop=(k == 0),
                    )
                pos[k] = po
                if k < NB - 1:
                    for h in range(2):
                        nc.tensor.matmul(
                            psS[h][:, :],
                            lhs_mv(k),
                            rhs_z(k, h),
                            start=(k == 0),
                            stop=(k == NB - 2),
                        )
                if k % 2 == 1:
                    c = k // 2
                    st = stp.tile([NB, U], bf16, tag="st")
                    nc.scalar.copy(st[:, 0:512], psS[0][0:NB, :])
                    nc.vector.tensor_copy(st[:, 512:U], psS[1][0:NB, :])
                    stiles[c] = st

            def emit_back(k):
                po = pos.pop(k)
                if k > 0:
                    # S row k-1 is covered by the copy after matvec of
                    # block b = k-1 if odd else k (chunk index b//2)
                    b = k - 1 if (k - 1) % 2 == 1 else k
                    st = stiles[b // 2]
                    for h in range(2):
                        nc.tensor.matmul(
                            po[h][:, :],
                            lhs_selt(k),
                            st[0:NB, h * 512 : h * 512 + 512],
                            start=False,
                            stop=True,
                        )
                if k % 2 == 0:
                    ot = otp.tile([L, 2 * U], bf16, tag="ot")
                    ots[k // 2] = ot
                ot = ots[k // 2]
                o0 = (k % 2) * U
                nc.vector.tensor_copy(ot[:, o0 : o0 + 512], po[0][:, :])
                nc.scalar.copy(ot[:, o0 + 512 : o0 + U], po[1][:, :])
                if k % 2 == 1:
                    d = k // 2
                    ot_full = ots.pop(d)
                    if d >= ND - 2:
                        nc.sync.dma_start(out[d, :, 0:U], ot_full[:, 0:U])
                        nc.sync.dma_start(out[d, :, U:], ot_full[:, U:])
                    else:
                        nc.sync.dma_start(out[d, :, :], ot_full[:])

            for w in range(NWARM):
                pw = pop.tile([L, 512], f32, tag="poa" if w % 2 == 0 else "pob")
                nc.tensor.matmul(
                    pw[:, :], warm[:, 0:L], warm[:, L : L + 512],
                    start=True, stop=True,
                )
            for k in range(NB):
                emit_front(k)
                if k >= DELAY:
                    emit_back(k - DELAY)
            for k in range(NB - DELAY, NB):
                emit_back(k)
    nc.finalize()
    return nc


_NC = None


def _get_nc() -> bass.Bass:
    global _NC
    if _NC is None:
        _NC = build_nc()
    return _NC


def prep_in_maps(x: np.ndarray) -> list[dict]:
    maps = []
    sidx = np.arange(L)
    smask = sidx[None, :, None] <= sidx[None, None, :]
    for c in range(B):
        xs = x[c, :, 0].astype(np.float64)
        ys = x[c, :, 1].astype(np.float64)
        z = x[c, :, 2:]
        yb = ys.reshape(NB, L)
        xb = xs.reshape(NB, L)
        cp = np.cumprod(yb, axis=1)  # cp[k,t] = prod_{r=0..t} y_{kL+r}
        ratio = cp[:, None, :] / cp[:, :, None]  # prod_{s+1..t}
        mt = xb[:, :, None] * ratio * smask  # [k, s, t]
        mlast = mt[:, :, L - 1]  # [k, s]
        A = cp[:, L - 1]
        W = np.zeros((NB, NB))
        for r in range(NB):
            W[r, r] = 1.0
            if r:
                W[r, :r] = W[r - 1, :r] * A[r]
        mv2 = mlast[:, :, None] * W.T[:, None, :]  # [k, s, r]
        selt = np.zeros((NB, NB, L))  # [s, k, t]
        for k in range(1, NB):
            selt[k - 1, k, :] = cp[k]

        # zin[p, k, c] = z[k*L + p, c]
        zb = (
            np.ascontiguousarray(z)
            .astype(nbf16)
            .reshape(NB, L, U)
            .transpose(1, 0, 2)
        )
        maps.append(
            {
                "zin": np.ascontiguousarray(zb),
                "mtT": np.ascontiguousarray(
                    mt.transpose(1, 0, 2).reshape(L, NB * L).astype(nbf16)
                ),
                "mvT": np.ascontiguousarray(
                    mv2.transpose(1, 0, 2).reshape(L, NB * NB).astype(nbf16)
                ),
                "seltT": np.ascontiguousarray(
                    selt.reshape(NB, NB * L).astype(nbf16)
                ),
            }
        )
    return maps


def unpack_out(outb: np.ndarray) -> np.ndarray:
    # outb [B, ND, L, 2U]: out[d, p, j*U+c] = res[(2d+j)L+p, c]
    o = outb.reshape(B, ND, L, 2, U)
    o = o.transpose(0, 1, 3, 2, 4)  # [B, d, j, p, c]
    return np.ascontiguousarray(o).reshape(B, T, U).astype(np.float32)


def kernel(**inputs: np.ndarray) -> np.ndarray:
    x = np.ascontiguousarray(inputs["inputs"], dtype=np.float32)
    assert x.shape == (B, T, F), x.shape
    nc = _get_nc()
    res = run_bass_kernel_spmd(nc, prep_in_maps(x), core_ids=list(range(B)))
    outb = np.stack([res.results[c]["out"] for c in range(B)], axis=0)
    return unpack_out(outb)


# revision 14
# speedup vs baseline: 1.1945x; 1.1379x over previous
"""Trainium2 Bass kernel for CLSProcess: diagonal linear recurrence
state_t = y_t * state_{t-1} + x_t * z_t over [B=8, T=4096, units=1024].

Sharding: batch across the 8 cores (one batch element per core).

v6 design — NO serial inter-block chain on device. The block recurrence
s_k = A_k s_{k-1} + b_k is solved with host-precomputed weights:

  - Host (f64, exact): per-block decay matrices M'_k[s,t] = x_s*prod(y)
    (main-matmul lhsT), the triangular inter-block propagator
    W[r,j] = prod_{i=j+1..r} A_i, "stacked matvec" weights
    mv2_j = outer(mlast_j, W[:,j]) and carry selectors
    selt_k[s,t] = I[s==k-1] * p_{k,t}. All shipped as bf16 sidecars.
  - Device (per core, single pass, N=1024 bf16 matmuls into 2-bank
    PSUM tiles):
      main:   po_k  = M'_k.T @ z_k           (start, no stop)
      matvec: ps_S += mv2_k.T @ z_k          (stacked: row r of ps_S
              accumulates W[r,k]*b_k, so S rows <= k are FINAL after
              matvec k — the inter-block scan happens inside PSUM
              accumulation, progressively, with no barrier)
      every 2 blocks: copy ps_S -> Stile (bf16, triple-buffered; two
              [32,512] copies split across scalar+vector)
      carry:  po_k += selt_k.T @ Stile       (K=32, start=False)
      drain:  one [128,1024] CAST -> bf16 out tile -> DMA out
  - Carries/drains are emitted DELAY=2 blocks behind mains; with CH=2
    the S-copy for row k-1 lands >=2 iterations before carry_k needs
    it, so the in-order PE queue never stalls on the S pipeline.
  - PSUM: 3 po tiles (2 banks each) + ps_S (2 banks) = 8 banks.
  - z is loaded via per-chunk SBUF tiles (sized small-first so block 0
    starts ~1us after the DMA preamble); M' sidecar is split in 4
    separate tiles so main_0 doesn't wait on the full 1MB transfer.
    I/O bf16; output written as [ND, 128, 2048] (2-block 4KB lines).
"""

import numpy as np
import ml_dtypes

import concourse.bacc as bacc
import concourse.bass as bass
import concourse.mybir as mybir
import concourse.tile as tile
from concourse.bass_utils import run_bass_kernel_spmd

B = 8
T = 4096
F = 1026
U = 1024
L = 128
NB = T // L  # 32 blocks
ND = NB // 2  # 16 two-block DMA groups
DELAY = 2  # carry/drain emission lag behind mains (blocks)
# z DMA chunking (in blocks): small chunks first so the pipeline
# starts early, large later to cut DMA instruction count
ZCHUNKS = (1, 1, 2, 2, 2, 4, 6, 7, 7)
NWARM = 6  # HAM warm-up dummy matmuls during the initial load phase
f32 = mybir.dt.float32
bf16 = mybir.dt.bfloat16
nbf16 = ml_dtypes.bfloat16


def build_nc() -> bass.Bass:
    nc = bacc.Bacc()
    # zin[p, k, c] = z_{k*L + p, c}  (per-p 64KB contiguous)
    zin = nc.dram_tensor("zin", [L, NB, U], bf16, kind="ExternalInput")
    # mtT[s, k*L + t] = M'_k[t, s] = x_{kL+s} * prod_{r=kL+s+1..kL+t} y_r
    mtT = nc.dram_tensor("mtT", [L, NB * L], bf16, kind="ExternalInput")
    # mvT[s, k*64 + 32i + r] = W[r, k] * M'_k[L-1, s]  (2 replicas)
    MVW = 2 * NB
    mvT = nc.dram_tensor("mvT", [L, NB * MVW], bf16, kind="ExternalInput")
    # seltT[32i + s, k*L + t] = I[s == k-1] * prod_{r=kL..kL+t} y_r
    seltT = nc.dram_tensor("seltT", [2 * NB, NB * L], bf16, kind="ExternalInput")
    # out[d, p, j*U + c] = out_{(2d+j)*L + p, c}
    out = nc.dram_tensor("out", [ND, L, 2 * U], bf16, kind="ExternalOutput")

    with tile.TileContext(nc) as tc:
        with (
            tc.tile_pool(name="const", bufs=1) as constp,
            tc.tile_pool(name="zpool", bufs=1) as zp,
            tc.tile_pool(name="stilep", bufs=3) as stp,
            tc.tile_pool(name="otpool", bufs=4) as otp,
            tc.tile_pool(name="po", bufs=3, space="PSUM") as pop,
            tc.tile_pool(name="psS", bufs=1, space="PSUM") as psp,
        ):
            # mt split so early blocks only wait on their own slice
            MTCH = (2, 6, 8, 8, 8)
            mts = []  # (tile, k0, nblocks)
            k0 = 0
            for qi, qw in enumerate(MTCH):
                t = constp.tile([L, qw * L], bf16, tag=f"mt{qi}", name=f"mt{qi}")
                mts.append((t, k0, qw))
                k0 += qw
            MVH = 4  # mv head blocks
            mvh = constp.tile([L, MVH * MVW], bf16, tag="mvh")
            mvt = constp.tile([L, (NB - MVH) * MVW], bf16, tag="mvt")
            SEH = 8  # selt head blocks
            seh = constp.tile([2 * NB, SEH * L], bf16, tag="seh")
            set_ = constp.tile([2 * NB, (NB - SEH) * L], bf16, tag="set")
            zts = []  # (tile, k0, nblocks)
            zmap = {}  # block k -> (tile, block offset in chunk)
            k0 = 0
            for ci, cw in enumerate(ZCHUNKS):
                zt = zp.tile([L, cw * U], bf16, tag=f"tz{ci}", name=f"tz{ci}")
                zts.append((zt, k0, cw))
                for kk in range(cw):
                    zmap[k0 + kk] = (zt, kk)
                k0 += cw

            # warm-up tile for HAM dummy matmuls (no DMA dependency)
            warm = constp.tile([L, 640], bf16, tag="warm")
            nc.vector.memset(warm[:], 0.0)

            # DMA emission order: first-needed first
            def zdma(i):
                zt, zk0, zw = zts[i]
                nc.sync.dma_start(zt[:], zin[:, zk0 : zk0 + zw, :])

            def mtdma(i):
                t, tk0, tw = mts[i]
                nc.sync.dma_start(t[:], mtT[:, tk0 * L : (tk0 + tw) * L])

            zdma(0)
            mtdma(0)
            nc.sync.dma_start(mvh[:], mvT[:, 0 : MVH * MVW])
            zdma(1)
            nc.sync.dma_start(seh[:], seltT[:, 0 : SEH * L])
            mtdma(1)
            zdma(2)
            zdma(3)
            mtdma(2)
            nc.sync.dma_start(mvt[:], mvT[:, MVH * MVW :])
            zdma(4)
            nc.sync.dma_start(set_[:], seltT[:, SEH * L :])
            mtdma(3)
            zdma(5)
            mtdma(4)
            zdma(6)
            zdma(7)
            zdma(8)

            def rhs_z(k, h):
                zt, kk = zmap[k]
                off = kk * U + h * 512
                return zt[:, off : off + 512]

            def lhs_mt(k):
                for t, tk0, tw in mts:
                    if tk0 <= k < tk0 + tw:
                        return t[:, (k - tk0) * L : (k - tk0 + 1) * L]
                raise AssertionError(k)

            def lhs_mv(k):
                if k < MVH:
                    return mvh[:, k * MVW : (k + 1) * MVW]
                return mvt[:, (k - MVH) * MVW : (k - MVH + 1) * MVW]

            def lhs_selt(k, q):
                p0 = 32 * q
                if k < SEH:
                    return seh[p0 : p0 + NB, k * L : (k + 1) * L]
                return set_[p0 : p0 + NB, (k - SEH) * L : (k - SEH + 1) * L]

            psS = (
                psp.tile([2 * NB, 512], f32, tag="psSa", name="psSa"),
                psp.tile([2 * NB, 512], f32, tag="psSb", name="psSb"),
            )
            stiles = {}
            pos = {}
            ots = {}

            def emit_front(k):
                po_a = pop.tile([L, 512], f32, tag="poa")
                po_b = pop.tile([L, 512], f32, tag="pob")
                po = (po_a, po_b)
                for h in range(2):
                    nc.tensor.matmul(
                        po[h][:, :],
                        lhs_mt(k),
                        rhs_z(k, h),
                        start=True,
                        stop=(k == 0),
                    )
                pos[k] = po
                if k < NB - 1:
                    for h in range(2):
                        nc.tensor.matmul(
                            psS[h][:, :],
                            lhs_mv(k),
                            rhs_z(k, h),
                            start=(k == 0),
                            stop=(k == NB - 2),
                        )
                if k % 2 == 1:
                    c = k // 2
                    st = stp.tile([2 * NB, U], bf16, tag="st")
                    nc.scalar.copy(st[:, 0:512], psS[0][0 : 2 * NB, :])
                    nc.vector.tensor_copy(st[:, 512:U], psS[1][0 : 2 * NB, :])
                    stiles[c] = st

            def emit_drain(k):
                po = pos.pop(k)
                if k % 2 == 0:
                    ot = otp.tile([L, 2 * U], bf16, tag="ot")
                    ots[k // 2] = ot
                ot = ots[k // 2]
                o0 = (k % 2) * U
                nc.vector.tensor_copy(ot[:, o0 : o0 + 512], po[0][:, :])
                nc.scalar.copy(ot[:, o0 + 512 : o0 + U], po[1][:, :])
                if k % 2 == 1:
                    d = k // 2
                    ot_full = ots.pop(d)
                    if d >= ND - 2:
                        nc.sync.dma_start(out[d, :, 0:U], ot_full[:, 0:U])
                        nc.sync.dma_start(out[d, :, U:], ot_full[:, U:])
                    else:
                        nc.sync.dma_start(out[d, :, :], ot_full[:])

            def emit_pack(blocks):
                # paired carries on disjoint 32-row PE groups (concurrent
                # via tile_position); S rows <= max(blocks)-1 are covered
                # by the Scopy of chunk (max-1)//2
                c = (max(blocks) - 1) // 2
                st = stiles[c]
                for h in range(2):
                    for q, k in enumerate(blocks):
                        p0 = 32 * q
                        nc.tensor.matmul(
                            pos[k][h][:, :],
                            lhs_selt(k, q),
                            st[p0 : p0 + NB, h * 512 : h * 512 + 512],
                            start=False,
                            stop=True,
                        )
                for k in blocks:
                    emit_drain(k)

            for w in range(NWARM):
                pw = pop.tile([L, 512], f32, tag="poa" if w % 2 == 0 else "pob")
                nc.tensor.matmul(
                    pw[:, :], warm[:, 0:L], warm[:, L : L + 512],
                    start=True, stop=True,
                )
            for k in range(NB):
                emit_front(k)
                if k == 1:
                    emit_drain(0)
                if k >= 3 and k % 2 == 1:
                    emit_pack((k - 2, k - 1))
            emit_pack((NB - 1,))
    nc.finalize()
    return nc


_NC = None


def _get_nc() -> bass.Bass:
    global _NC
    if _NC is None:
        _NC = build_nc()
    return _NC


def prep_in_maps(x: np.ndarray) -> list[dict]:
    maps = []
    sidx = np.arange(L)
    smask = sidx[None, :, None] <= sidx[None, None, :]
    for c in range(B):
        xs = x[c, :, 0].astype(np.float64)
        ys = x[c, :, 1].astype(np.float64)
        z = x[c, :, 2:]
        yb = ys.reshape(NB, L)
        xb = xs.reshape(NB, L)
        cp = np.cumprod(yb, axis=1)  # cp[k,t] = prod_{r=0..t} y_{kL+r}
        ratio = cp[:, None, :] / cp[:, :, None]  # prod_{s+1..t}
        mt = xb[:, :, None] * ratio * smask  # [k, s, t]
        mlast = mt[:, :, L - 1]  # [k, s]
        A = cp[:, L - 1]
        W = np.zeros((NB, NB))
        for r in range(NB):
            W[r, r] = 1.0
            if r:
                W[r, :r] = W[r - 1, :r] * A[r]
        mv2 = mlast[:, :, None] * W.T[:, None, :]  # [k, s, r]
        mv2 = np.concatenate([mv2, mv2], axis=2)  # [k, s, 64] 2 replicas
        selt = np.zeros((NB, NB, L))  # [s, k, t]
        for k in range(1, NB):
            selt[k - 1, k, :] = cp[k]
        selt = np.concatenate([selt, selt], axis=0)  # [64, k, t]

        # zin[p, k, c] = z[k*L + p, c]
        zb = (
            np.ascontiguousarray(z)
            .astype(nbf16)
            .reshape(NB, L, U)
            .transpose(1, 0, 2)
        )
        maps.append(
            {
                "zin": np.ascontiguousarray(zb),
                "mtT": np.ascontiguousarray(
                    mt.transpose(1, 0, 2).reshape(L, NB * L).astype(nbf16)
                ),
                "mvT": np.ascontiguousarray(
                    mv2.transpose(1, 0, 2).reshape(L, NB * 2 * NB).astype(nbf16)
                ),
                "seltT": np.ascontiguousarray(
                    selt.reshape(2 * NB, NB * L).astype(nbf16)
                ),
            }
        )
    return maps


def unpack_out(outb: np.ndarray) -> np.ndarray:
    # outb [B, ND, L, 2U]: out[d, p, j*U+c] = res[(2d+j)L+p, c]
    o = outb.reshape(B, ND, L, 2, U)
    o = o.transpose(0, 1, 3, 2, 4)  # [B, d, j, p, c]
    return np.ascontiguousarray(o).reshape(B, T, U).astype(np.float32)


def kernel(**inputs: np.ndarray) -> np.ndarray:
    x = np.ascontiguousarray(inputs["inputs"], dtype=np.float32)
    assert x.shape == (B, T, F), x.shape
    nc = _get_nc()
    res = run_bass_kernel_spmd(nc, prep_in_maps(x), core_ids=list(range(B)))
    outb = np.stack([res.results[c]["out"] for c in range(B)], axis=0)
    return unpack_out(outb)


# revision 15
# speedup vs baseline: 1.2328x; 1.0321x over previous
"""Trainium2 Bass kernel for CLSProcess: diagonal linear recurrence
state_t = y_t * state_{t-1} + x_t * z_t over [B=8, T=4096, units=1024].

Sharding: batch across the 8 cores (one batch element per core).

v6 design — NO serial inter-block chain on device. The block recurrence
s_k = A_k s_{k-1} + b_k is solved with host-precomputed weights:

  - Host (f64, exact): per-block decay matrices M'_k[s,t] = x_s*prod(y)
    (main-matmul lhsT), the triangular inter-block propagator
    W[r,j] = prod_{i=j+1..r} A_i, "stacked matvec" weights
    mv2_j = outer(mlast_j, W[:,j]) and carry selectors
    selt_k[s,t] = I[s==k-1] * p_{k,t}. All shipped as bf16 sidecars.
  - Device (per core, single pass, N=1024 bf16 matmuls into 2-bank
    PSUM tiles):
      main:   po_k  = M'_k.T @ z_k           (start, no stop)
      matvec: ps_S += mv2_k.T @ z_k          (stacked: row r of ps_S
              accumulates W[r,k]*b_k, so S rows <= k are FINAL after
              matvec k — the inter-block scan happens inside PSUM
              accumulation, progressively, with no barrier)
      every 2 blocks: copy ps_S -> Stile (bf16, triple-buffered; two
              [32,512] copies split across scalar+vector)
      carry:  po_k += selt_k.T @ Stile       (K=32, start=False)
      drain:  one [128,1024] CAST -> bf16 out tile -> DMA out
  - Carries/drains are emitted DELAY=2 blocks behind mains; with CH=2
    the S-copy for row k-1 lands >=2 iterations before carry_k needs
    it, so the in-order PE queue never stalls on the S pipeline.
  - PSUM: 3 po tiles (2 banks each) + ps_S (2 banks) = 8 banks.
  - z is loaded via per-chunk SBUF tiles (sized small-first so block 0
    starts ~1us after the DMA preamble); M' sidecar is split in 4
    separate tiles so main_0 doesn't wait on the full 1MB transfer.
    I/O bf16; output written as [ND, 128, 2048] (2-block 4KB lines).
"""

import numpy as np
import ml_dtypes

import concourse.bacc as bacc
import concourse.bass as bass
import concourse.mybir as mybir
import concourse.tile as tile
from concourse.bass_utils import run_bass_kernel_spmd

B = 8
T = 4096
F = 1026
U = 1024
L = 128
NB = T // L  # 32 blocks
ND = NB // 2  # 16 two-block DMA groups
DELAY = 2  # carry/drain emission lag behind mains (blocks)
# z DMA chunking (in blocks): small chunks first so the pipeline
# starts early, large later to cut DMA instruction count
ZCHUNKS = (1, 1, 1, 2, 2, 2, 3, 4, 4, 4, 4, 4)
NWARM = 6  # HAM warm-up dummy matmuls during the initial load phase
f32 = mybir.dt.float32
bf16 = mybir.dt.bfloat16
nbf16 = ml_dtypes.bfloat16


def build_nc() -> bass.Bass:
    nc = bacc.Bacc()
    # zin[p, k, c] = z_{k*L + p, c}  (per-p 64KB contiguous)
    zin = nc.dram_tensor("zin", [L, NB, U], bf16, kind="ExternalInput")
    # mtT[s, k*L + t] = M'_k[t, s] = x_{kL+s} * prod_{r=kL+s+1..kL+t} y_r
    mtT = nc.dram_tensor("mtT", [L, NB * L], bf16, kind="ExternalInput")
    # mvT[s, k*64 + 32i + r] = W[r, k] * M'_k[L-1, s]  (2 replicas)
    MVW = 2 * NB
    mvT = nc.dram_tensor("mvT", [L, NB * MVW], bf16, kind="ExternalInput")
    # seltT[32i + s, k*L + t] = I[s == k-1] * prod_{r=kL..kL+t} y_r
    seltT = nc.dram_tensor("seltT", [2 * NB, NB * L], bf16, kind="ExternalInput")
    # out[d, p, j*U + c] = out_{(2d+j)*L + p, c}
    out = nc.dram_tensor("out", [ND, L, 2 * U], bf16, kind="ExternalOutput")

    with tile.TileContext(nc) as tc:
        with (
            tc.tile_pool(name="const", bufs=1) as constp,
            tc.tile_pool(name="zpool", bufs=1) as zp,
            tc.tile_pool(name="stilep", bufs=3) as stp,
            tc.tile_pool(name="otpool", bufs=6) as otp,
            tc.tile_pool(name="po", bufs=3, space="PSUM") as pop,
            tc.tile_pool(name="psS", bufs=1, space="PSUM") as psp,
        ):
            # mt split so early blocks only wait on their own slice
            MTCH = (2, 6, 8, 8, 8)
            mts = []  # (tile, k0, nblocks)
            k0 = 0
            for qi, qw in enumerate(MTCH):
                t = constp.tile([L, qw * L], bf16, tag=f"mt{qi}", name=f"mt{qi}")
                mts.append((t, k0, qw))
                k0 += qw
            MVH = 4  # mv head blocks
            mvh = constp.tile([L, MVH * MVW], bf16, tag="mvh")
            mvt = constp.tile([L, (NB - MVH) * MVW], bf16, tag="mvt")
            SEH = 8  # selt head blocks
            seh = constp.tile([2 * NB, SEH * L], bf16, tag="seh")
            set_ = constp.tile([2 * NB, (NB - SEH) * L], bf16, tag="set")
            zts = []  # (tile, k0, nblocks)
            zmap = {}  # block k -> (tile, block offset in chunk)
            k0 = 0
            for ci, cw in enumerate(ZCHUNKS):
                zt = zp.tile([L, cw * U], bf16, tag=f"tz{ci}", name=f"tz{ci}")
                zts.append((zt, k0, cw))
                for kk in range(cw):
                    zmap[k0 + kk] = (zt, kk)
                k0 += cw

            # warm-up tile for HAM dummy matmuls (no DMA dependency)
            warm = constp.tile([L, 640], bf16, tag="warm")
            nc.vector.memset(warm[:], 0.0)

            # DMA emission order: first-needed first
            def zdma(i):
                zt, zk0, zw = zts[i]
                nc.sync.dma_start(zt[:], zin[:, zk0 : zk0 + zw, :])

            def mtdma(i):
                t, tk0, tw = mts[i]
                nc.sync.dma_start(t[:], mtT[:, tk0 * L : (tk0 + tw) * L])

            zdma(0)
            mtdma(0)
            nc.sync.dma_start(mvh[:], mvT[:, 0 : MVH * MVW])
            zdma(1)
            nc.sync.dma_start(seh[:], seltT[:, 0 : SEH * L])
            mtdma(1)
            zdma(2)
            zdma(3)
            mtdma(2)
            nc.sync.dma_start(mvt[:], mvT[:, MVH * MVW :])
            zdma(4)
            nc.sync.dma_start(set_[:], seltT[:, SEH * L :])
            zdma(5)
            mtdma(3)
            zdma(6)
            mtdma(4)
            zdma(7)
            zdma(8)
            zdma(9)
            zdma(10)
            zdma(11)

            def rhs_z(k, h):
                zt, kk = zmap[k]
                off = kk * U + h * 512
                return zt[:, off : off + 512]

            def lhs_mt(k):
                for t, tk0, tw in mts:
                    if tk0 <= k < tk0 + tw:
                        return t[:, (k - tk0) * L : (k - tk0 + 1) * L]
                raise AssertionError(k)

            def lhs_mv(k):
                if k < MVH:
                    return mvh[:, k * MVW : (k + 1) * MVW]
                return mvt[:, (k - MVH) * MVW : (k - MVH + 1) * MVW]

            def lhs_selt(k, q):
                p0 = 32 * q
                if k < SEH:
                    return seh[p0 : p0 + NB, k * L : (k + 1) * L]
                return set_[p0 : p0 + NB, (k - SEH) * L : (k - SEH + 1) * L]

            psS = (
                psp.tile([2 * NB, 512], f32, tag="psSa", name="psSa"),
                psp.tile([2 * NB, 512], f32, tag="psSb", name="psSb"),
            )
            stiles = {}
            pos = {}
            ots = {}

            def emit_front(k):
                po_a = pop.tile([L, 512], f32, tag="poa")
                po_b = pop.tile([L, 512], f32, tag="pob")
                po = (po_a, po_b)
                for h in range(2):
                    nc.tensor.matmul(
                        po[h][:, :],
                        lhs_mt(k),
                        rhs_z(k, h),
                        start=True,
                        stop=(k == 0),
                    )
                pos[k] = po
                if k < NB - 1:
                    for h in range(2):
                        nc.tensor.matmul(
                            psS[h][:, :],
                            lhs_mv(k),
                            rhs_z(k, h),
                            start=(k == 0),
                            stop=(k == NB - 2),
                        )
                if k % 2 == 1:
                    c = k // 2
                    st = stp.tile([2 * NB, U], bf16, tag="st")
                    nc.scalar.copy(st[:, 0:512], psS[0][0 : 2 * NB, :])
                    nc.vector.tensor_copy(st[:, 512:U], psS[1][0 : 2 * NB, :])
                    stiles[c] = st

            def emit_drain(k):
                po = pos.pop(k)
                if k % 2 == 0:
                    ot = otp.tile([L, 2 * U], bf16, tag="ot")
                    ots[k // 2] = ot
                ot = ots[k // 2]
                o0 = (k % 2) * U
                nc.vector.tensor_copy(ot[:, o0 : o0 + 512], po[0][:, :])
                nc.scalar.copy(ot[:, o0 + 512 : o0 + U], po[1][:, :])
                if k % 2 == 1:
                    d = k // 2
                    ot_full = ots.pop(d)
                    if d >= ND - 2:
                        nc.sync.dma_start(out[d, :, 0:U], ot_full[:, 0:U])
                        nc.sync.dma_start(out[d, :, U:], ot_full[:, U:])
                    else:
                        nc.sync.dma_start(out[d, :, :], ot_full[:])

            def emit_pack(blocks):
                # paired carries on disjoint 32-row PE groups (concurrent
                # via tile_position); S rows <= max(blocks)-1 are covered
                # by the Scopy of chunk (max-1)//2
                c = (max(blocks) - 1) // 2
                st = stiles[c]
                for h in range(2):
                    for q, k in enumerate(blocks):
                        p0 = 32 * q
                        nc.tensor.matmul(
                            pos[k][h][:, :],
                            lhs_selt(k, q),
                            st[p0 : p0 + NB, h * 512 : h * 512 + 512],
                            start=False,
                            stop=True,
                        )
                for k in blocks:
                    emit_drain(k)

            for w in range(NWARM):
                pw = pop.tile([L, 512], f32, tag="poa" if w % 2 == 0 else "pob")
                nc.tensor.matmul(
                    pw[:, :], warm[:, 0:L], warm[:, L : L + 512],
                    start=True, stop=True,
                )
            for k in range(NB):
                emit_front(k)
                if k == 1:
                    emit_drain(0)
                if k >= 3 and k % 2 == 1:
                    emit_pack((k - 2, k - 1))
            emit_pack((NB - 1,))
    nc.finalize()
    return nc


_NC = None


def _get_nc() -> bass.Bass:
    global _NC
    if _NC is None:
        _NC = build_nc()
    return _NC


def prep_in_maps(x: np.ndarray) -> list[dict]:
    maps = []
    sidx = np.arange(L)
    smask = sidx[None, :, None] <= sidx[None, None, :]
    for c in range(B):
        xs = x[c, :, 0].astype(np.float64)
        ys = x[c, :, 1].astype(np.float64)
        z = x[c, :, 2:]
        yb = ys.reshape(NB, L)
        xb = xs.reshape(NB, L)
        cp = np.cumprod(yb, axis=1)  # cp[k,t] = prod_{r=0..t} y_{kL+r}
        ratio = cp[:, None, :] / cp[:, :, None]  # prod_{s+1..t}
        mt = xb[:, :, None] * ratio * smask  # [k, s, t]
        mlast = mt[:, :, L - 1]  # [k, s]
        A = cp[:, L - 1]
        W = np.zeros((NB, NB))
        for r in range(NB):
            W[r, r] = 1.0
            if r:
                W[r, :r] = W[r - 1, :r] * A[r]
        mv2 = mlast[:, :, None] * W.T[:, None, :]  # [k, s, r]
        mv2 = np.concatenate([mv2, mv2], axis=2)  # [k, s, 64] 2 replicas
        selt = np.zeros((NB, NB, L))  # [s, k, t]
        for k in range(1, NB):
            selt[k - 1, k, :] = cp[k]
        selt = np.concatenate([selt, selt], axis=0)  # [64, k, t]

        # zin[p, k, c] = z[k*L + p, c]
        zb = (
            np.ascontiguousarray(z)
            .astype(nbf16)
            .reshape(NB, L, U)
            .transpose(1, 0, 2)
        )
        maps.append(
            {
                "zin": np.ascontiguousarray(zb),
                "mtT": np.ascontiguousarray(
                    mt.transpose(1, 0, 2).reshape(L, NB * L).astype(nbf16)
                ),
                "mvT": np.ascontiguousarray(
                    mv2.transpose(1, 0, 2).reshape(L, NB * 2 * NB).astype(nbf16)
                ),
                "seltT": np.ascontiguousarray(
                    selt.reshape(2 * NB, NB * L).astype(nbf16)
                ),
            }
        )
    return maps


def unpack_out(outb: np.ndarray) -> np.ndarray:
    # outb [B, ND, L, 2U]: out[d, p, j*U+c] = res[(2d+j)L+p, c]
    o = outb.reshape(B, ND, L, 2, U)
    o = o.transpose(0, 1, 3, 2, 4)  # [B, d, j, p, c]
    return np.ascontiguousarray(o).reshape(B, T, U).astype(np.float32)


def kernel(**inputs: np.ndarray) -> np.ndarray:
    x = np.ascontiguousarray(inputs["inputs"], dtype=np.float32)
    assert x.shape == (B, T, F), x.shape
    nc = _get_nc()
    res = run_bass_kernel_spmd(nc, prep_in_maps(x), core_ids=list(range(B)))
    outb = np.stack([res.results[c]["out"] for c in range(B)], axis=0)
    return unpack_out(outb)


# revision 16
# speedup vs baseline: 1.2853x; 1.0425x over previous
"""Trainium2 Bass kernel for CLSProcess: diagonal linear recurrence
state_t = y_t * state_{t-1} + x_t * z_t over [B=8, T=4096, units=1024].

Sharding: batch across the 8 cores (one batch element per core).

v6 design — NO serial inter-block chain on device. The block recurrence
s_k = A_k s_{k-1} + b_k is solved with host-precomputed weights:

  - Host (f64, exact): per-block decay matrices M'_k[s,t] = x_s*prod(y)
    (main-matmul lhsT), the triangular inter-block propagator
    W[r,j] = prod_{i=j+1..r} A_i, "stacked matvec" weights
    mv2_j = outer(mlast_j, W[:,j]) and carry selectors
    selt_k[s,t] = I[s==k-1] * p_{k,t}. All shipped as bf16 sidecars.
  - Device (per core, single pass, N=1024 bf16 matmuls into 2-bank
    PSUM tiles):
      main:   po_k  = M'_k.T @ z_k           (start, no stop)
      matvec: ps_S += mv2_k.T @ z_k          (stacked: row r of ps_S
              accumulates W[r,k]*b_k, so S rows <= k are FINAL after
              matvec k — the inter-block scan happens inside PSUM
              accumulation, progressively, with no barrier)
      every 2 blocks: copy ps_S -> Stile (bf16, triple-buffered; two
              [32,512] copies split across scalar+vector)
      carry:  po_k += selt_k.T @ Stile       (K=32, start=False)
      drain:  one [128,1024] CAST -> bf16 out tile -> DMA out
  - Carries/drains are emitted DELAY=2 blocks behind mains; with CH=2
    the S-copy for row k-1 lands >=2 iterations before carry_k needs
    it, so the in-order PE queue never stalls on the S pipeline.
  - PSUM: 3 po tiles (2 banks each) + ps_S (2 banks) = 8 banks.
  - z is loaded via per-chunk SBUF tiles (sized small-first so block 0
    starts ~1us after the DMA preamble); M' sidecar is split in 4
    separate tiles so main_0 doesn't wait on the full 1MB transfer.
    I/O bf16; output written as [ND, 128, 2048] (2-block 4KB lines).
"""

import numpy as np
import ml_dtypes

import concourse.bacc as bacc
import concourse.bass as bass
import concourse.mybir as mybir
import concourse.tile as tile
from concourse.bass_utils import run_bass_kernel_spmd

B = 8
T = 4096
F = 1026
U = 1024
L = 128
NB = T // L  # 32 blocks
ND = NB // 2  # 16 two-block DMA groups
DELAY = 2  # carry/drain emission lag behind mains (blocks)
# z DMA chunking (in blocks): small chunks first so the pipeline
# starts early, large later to cut DMA instruction count
ZCHUNKS = (1, 1, 1, 2, 2, 2, 3, 4, 4, 4, 4, 4)
NWARM = 12  # HAM warm-up dummy matmuls during the initial load phase
f32 = mybir.dt.float32
bf16 = mybir.dt.bfloat16
nbf16 = ml_dtypes.bfloat16


def build_nc() -> bass.Bass:
    nc = bacc.Bacc()
    # zin[p, k, c] = z_{k*L + p, c}  (per-p 64KB contiguous)
    zin = nc.dram_tensor("zin", [L, NB, U], bf16, kind="ExternalInput")
    # mtT[s, k*L + t] = M'_k[t, s] = x_{kL+s} * prod_{r=kL+s+1..kL+t} y_r
    mtT = nc.dram_tensor("mtT", [L, NB * L], bf16, kind="ExternalInput")
    # mvT[s, k*64 + 32i + r] = W[r, k] * M'_k[L-1, s]  (2 replicas)
    MVW = 2 * NB
    mvT = nc.dram_tensor("mvT", [L, NB * MVW], bf16, kind="ExternalInput")
    # seltT[32i + s, k*L + t] = I[s == k-1] * prod_{r=kL..kL+t} y_r
    seltT = nc.dram_tensor("seltT", [2 * NB, NB * L], bf16, kind="ExternalInput")
    # out[d, p, j*U + c] = out_{(2d+j)*L + p, c}
    out = nc.dram_tensor("out", [ND, L, 2 * U], bf16, kind="ExternalOutput")

    with tile.TileContext(nc) as tc:
        with (
            tc.tile_pool(name="const", bufs=1) as constp,
            tc.tile_pool(name="zpool", bufs=1) as zp,
            tc.tile_pool(name="stilep", bufs=3) as stp,
            tc.tile_pool(name="otpool", bufs=6) as otp,
            tc.tile_pool(name="po", bufs=3, space="PSUM") as pop,
            tc.tile_pool(name="psS", bufs=1, space="PSUM") as psp,
        ):
            # mt split so early blocks only wait on their own slice
            MTCH = (2, 6, 8, 8, 8)
            mts = []  # (tile, k0, nblocks)
            k0 = 0
            for qi, qw in enumerate(MTCH):
                t = constp.tile([L, qw * L], bf16, tag=f"mt{qi}", name=f"mt{qi}")
                mts.append((t, k0, qw))
                k0 += qw
            MVH = 4  # mv head blocks
            mvh = constp.tile([L, MVH * MVW], bf16, tag="mvh")
            mvt = constp.tile([L, (NB - MVH) * MVW], bf16, tag="mvt")
            SEH = 8  # selt head blocks
            seh = constp.tile([2 * NB, SEH * L], bf16, tag="seh")
            set_ = constp.tile([2 * NB, (NB - SEH) * L], bf16, tag="set")
            zts = []  # (tile, k0, nblocks)
            zmap = {}  # block k -> (tile, block offset in chunk)
            k0 = 0
            for ci, cw in enumerate(ZCHUNKS):
                zt = zp.tile([L, cw * U], bf16, tag=f"tz{ci}", name=f"tz{ci}")
                zts.append((zt, k0, cw))
                for kk in range(cw):
                    zmap[k0 + kk] = (zt, kk)
                k0 += cw

            # warm-up tile for HAM dummy matmuls (no DMA dependency)
            warm = constp.tile([L, 640], bf16, tag="warm")
            nc.vector.memset(warm[:], 0.0)

            # DMA emission order: first-needed first
            def zdma(i):
                zt, zk0, zw = zts[i]
                nc.sync.dma_start(zt[:], zin[:, zk0 : zk0 + zw, :])

            def mtdma(i):
                t, tk0, tw = mts[i]
                nc.sync.dma_start(t[:], mtT[:, tk0 * L : (tk0 + tw) * L])

            zdma(0)
            mtdma(0)
            nc.sync.dma_start(mvh[:], mvT[:, 0 : MVH * MVW])
            zdma(1)
            nc.sync.dma_start(seh[:], seltT[:, 0 : SEH * L])
            mtdma(1)
            zdma(2)
            zdma(3)
            mtdma(2)
            nc.sync.dma_start(mvt[:], mvT[:, MVH * MVW :])
            zdma(4)
            nc.sync.dma_start(set_[:], seltT[:, SEH * L :])
            zdma(5)
            mtdma(3)
            zdma(6)
            mtdma(4)
            zdma(7)
            zdma(8)
            zdma(9)
            zdma(10)
            zdma(11)

            def rhs_z(k, h):
                zt, kk = zmap[k]
                off = kk * U + h * 512
                return zt[:, off : off + 512]

            def lhs_mt(k):
                for t, tk0, tw in mts:
                    if tk0 <= k < tk0 + tw:
                        return t[:, (k - tk0) * L : (k - tk0 + 1) * L]
                raise AssertionError(k)

            def lhs_mv(k):
                if k < MVH:
                    return mvh[:, k * MVW : (k + 1) * MVW]
                return mvt[:, (k - MVH) * MVW : (k - MVH + 1) * MVW]

            def lhs_selt(k, q):
                p0 = 32 * q
                if k < SEH:
                    return seh[p0 : p0 + NB, k * L : (k + 1) * L]
                return set_[p0 : p0 + NB, (k - SEH) * L : (k - SEH + 1) * L]

            psS = (
                psp.tile([2 * NB, 512], f32, tag="psSa", name="psSa"),
                psp.tile([2 * NB, 512], f32, tag="psSb", name="psSb"),
            )
            stiles = {}
            pos = {}
            ots = {}

            def emit_front(k):
                po_a = pop.tile([L, 512], f32, tag="poa")
                po_b = pop.tile([L, 512], f32, tag="pob")
                po = (po_a, po_b)
                for h in range(2):
                    nc.tensor.matmul(
                        po[h][:, :],
                        lhs_mt(k),
                        rhs_z(k, h),
                        start=True,
                        stop=(k == 0),
                    )
                pos[k] = po
                if k < NB - 1:
                    for h in range(2):
                        nc.tensor.matmul(
                            psS[h][:, :],
                            lhs_mv(k),
                            rhs_z(k, h),
                            start=(k == 0),
                            stop=(k == NB - 2),
                        )
                if k % 2 == 1:
                    c = k // 2
                    st = stp.tile([2 * NB, U], bf16, tag="st")
                    nc.scalar.copy(st[:, 0:512], psS[0][0 : 2 * NB, :])
                    nc.vector.tensor_copy(st[:, 512:U], psS[1][0 : 2 * NB, :])
                    stiles[c] = st

            def emit_drain(k):
                po = pos.pop(k)
                if k % 2 == 0:
                    ot = otp.tile([L, 2 * U], bf16, tag="ot")
                    ots[k // 2] = ot
                ot = ots[k // 2]
                o0 = (k % 2) * U
                nc.vector.tensor_copy(ot[:, o0 : o0 + 512], po[0][:, :])
                nc.scalar.copy(ot[:, o0 + 512 : o0 + U], po[1][:, :])
                if k % 2 == 1:
                    d = k // 2
                    ot_full = ots.pop(d)
                    if d >= ND - 2:
                        nc.sync.dma_start(out[d, :, 0:U], ot_full[:, 0:U])
                        nc.sync.dma_start(out[d, :, U:], ot_full[:, U:])
                    else:
                        nc.sync.dma_start(out[d, :, :], ot_full[:])

            def emit_pack(blocks):
                # paired carries on disjoint 32-row PE groups (concurrent
                # via tile_position); S rows <= max(blocks)-1 are covered
                # by the Scopy of chunk (max-1)//2
                c = (max(blocks) - 1) // 2
                st = stiles[c]
                for h in range(2):
                    for q, k in enumerate(blocks):
                        p0 = 32 * q
                        nc.tensor.matmul(
                            pos[k][h][:, :],
                            lhs_selt(k, q),
                            st[p0 : p0 + NB, h * 512 : h * 512 + 512],
                            start=False,
                            stop=True,
                        )
                for k in blocks:
                    emit_drain(k)

            for w in range(NWARM):
                pw = pop.tile([L, 512], f32, tag="poa" if w % 2 == 0 else "pob")
                nc.tensor.matmul(
                    pw[:, :], warm[:, 0:L], warm[:, L : L + 512],
                    start=True, stop=True,
                )
            for k in range(NB):
                emit_front(k)
                if k == 1:
                    emit_drain(0)
                if k >= 3 and k % 2 == 1:
                    emit_pack((k - 2, k - 1))
            emit_pack((NB - 1,))
    nc.finalize()
    return nc


_NC = None


def _get_nc() -> bass.Bass:
    global _NC
    if _NC is None:
        _NC = build_nc()
    return _NC


def prep_in_maps(x: np.ndarray) -> list[dict]:
    maps = []
    sidx = np.arange(L)
    smask = sidx[None, :, None] <= sidx[None, None, :]
    for c in range(B):
        xs = x[c, :, 0].astype(np.float64)
        ys = x[c, :, 1].astype(np.float64)
        z = x[c, :, 2:]
        yb = ys.reshape(NB, L)
        xb = xs.reshape(NB, L)
        cp = np.cumprod(yb, axis=1)  # cp[k,t] = prod_{r=0..t} y_{kL+r}
        ratio = cp[:, None, :] / cp[:, :, None]  # prod_{s+1..t}
        mt = xb[:, :, None] * ratio * smask  # [k, s, t]
        mlast = mt[:, :, L - 1]  # [k, s]
        A = cp[:, L - 1]
        W = np.zeros((NB, NB))
        for r in range(NB):
            W[r, r] = 1.0
            if r:
                W[r, :r] = W[r - 1, :r] * A[r]
        mv2 = mlast[:, :, None] * W.T[:, None, :]  # [k, s, r]
        mv2 = np.concatenate([mv2, mv2], axis=2)  # [k, s, 64] 2 replicas
        selt = np.zeros((NB, NB, L))  # [s, k, t]
        for k in range(1, NB):
            selt[k - 1, k, :] = cp[k]
        selt = np.concatenate([selt, selt], axis=0)  # [64, k, t]

        # zin[p, k, c] = z[k*L + p, c]
        zb = (
            np.ascontiguousarray(z)
            .astype(nbf16)
            .reshape(NB, L, U)
            .transpose(1, 0, 2)
        )
        maps.append(
            {
                "zin": np.ascontiguousarray(zb),
                "mtT": np.ascontiguousarray(
                    mt.transpose(1, 0, 2).reshape(L, NB * L).astype(nbf16)
                ),
                "mvT": np.ascontiguousarray(
                    mv2.transpose(1, 0, 2).reshape(L, NB * 2 * NB).astype(nbf16)
                ),
                "seltT": np.ascontiguousarray(
                    selt.reshape(2 * NB, NB * L).astype(nbf16)
                ),
            }
        )
    return maps


def unpack_out(outb: np.ndarray) -> np.ndarray:
    # outb [B, ND, L, 2U]: out[d, p, j*U+c] = res[(2d+j)L+p, c]
    o = outb.reshape(B, ND, L, 2, U)
    o = o.transpose(0, 1, 3, 2, 4)  # [B, d, j, p, c]
    return np.ascontiguousarray(o).reshape(B, T, U).astype(np.float32)


def kernel(**inputs: np.ndarray) -> np.ndarray:
    x = np.ascontiguousarray(inputs["inputs"], dtype=np.float32)
    assert x.shape == (B, T, F), x.shape
    nc = _get_nc()
    res = run_bass_kernel_spmd(nc, prep_in_maps(x), core_ids=list(range(B)))
    outb = np.stack([res.results[c]["out"] for c in range(B)], axis=0)
    return unpack_out(outb)


# revision 17
# speedup vs baseline: 1.3726x; 1.0680x over previous
"""Trainium2 Bass kernel for CLSProcess: diagonal linear recurrence
state_t = y_t * state_{t-1} + x_t * z_t over [B=8, T=4096, units=1024].

Sharding: batch across the 8 cores (one batch element per core).

v6 design — NO serial inter-block chain on device. The block recurrence
s_k = A_k s_{k-1} + b_k is solved with host-precomputed weights:

  - Host (f64, exact): per-block decay matrices M'_k[s,t] = x_s*prod(y)
    (main-matmul lhsT), the triangular inter-block propagator
    W[r,j] = prod_{i=j+1..r} A_i, "stacked matvec" weights
    mv2_j = outer(mlast_j, W[:,j]) and carry selectors
    selt_k[s,t] = I[s==k-1] * p_{k,t}. All shipped as bf16 sidecars.
  - Device (per core, single pass, N=1024 bf16 matmuls into 2-bank
    PSUM tiles):
      main:   po_k  = M'_k.T @ z_k           (start, no stop)
      matvec: ps_S += mv2_k.T @ z_k          (stacked: row r of ps_S
              accumulates W[r,k]*b_k, so S rows <= k are FINAL after
              matvec k — the inter-block scan happens inside PSUM
              accumulation, progressively, with no barrier)
      every 2 blocks: copy ps_S -> Stile (bf16, triple-buffered; two
              [32,512] copies split across scalar+vector)
      carry:  po_k += selt_k.T @ Stile       (K=32, start=False)
      drain:  one [128,1024] CAST -> bf16 out tile -> DMA out
  - Carries/drains are emitted DELAY=2 blocks behind mains; with CH=2
    the S-copy for row k-1 lands >=2 iterations before carry_k needs
    it, so the in-order PE queue never stalls on the S pipeline.
  - PSUM: 3 po tiles (2 banks each) + ps_S (2 banks) = 8 banks.
  - z is loaded via per-chunk SBUF tiles (sized small-first so block 0
    starts ~1us after the DMA preamble); M' sidecar is split in 4
    separate tiles so main_0 doesn't wait on the full 1MB transfer.
    I/O bf16; output written as [ND, 128, 2048] (2-block 4KB lines).
"""

import numpy as np
import ml_dtypes

import concourse.bacc as bacc
import concourse.bass as bass
import concourse.mybir as mybir
import concourse.tile as tile
from concourse.bass_utils import run_bass_kernel_spmd

B = 8
T = 4096
F = 1026
U = 1024
L = 128
NB = T // L  # 32 blocks
ND = NB // 2  # 16 two-block DMA groups
DELAY = 2  # carry/drain emission lag behind mains (blocks)
# z DMA chunking (in blocks): small chunks first so the pipeline
# starts early, large later to cut DMA instruction count
ZCHUNKS = (1, 1, 1, 2, 2, 2, 3, 4, 4, 4, 4, 4)
NWARM = 6  # HAM warm-up dummy matmuls during the initial load phase
f32 = mybir.dt.float32
bf16 = mybir.dt.bfloat16
nbf16 = ml_dtypes.bfloat16


def build_nc() -> bass.Bass:
    nc = bacc.Bacc()
    # zin[p, k, c] = z_{k*L + p, c}  (per-p 64KB contiguous)
    zin = nc.dram_tensor("zin", [L, NB, U], bf16, kind="ExternalInput")
    # mtT[s, k*L + t] = M'_k[t, s] = x_{kL+s} * prod_{r=kL+s+1..kL+t} y_r
    mtT = nc.dram_tensor("mtT", [L, NB * L], bf16, kind="ExternalInput")
    # mvT[s, k*64 + 32i + r] = W[r, k] * M'_k[L-1, s]  (2 replicas)
    MVW = 2 * NB
    mvT = nc.dram_tensor("mvT", [L, NB * MVW], bf16, kind="ExternalInput")
    # seltT[32i + s, k*L + t] = I[s == k-1] * prod_{r=kL..kL+t} y_r
    seltT = nc.dram_tensor("seltT", [2 * NB, NB * L], bf16, kind="ExternalInput")
    # out[d, p, j*U + c] = out_{(2d+j)*L + p, c}
    out = nc.dram_tensor("out", [ND, L, 2 * U], bf16, kind="ExternalOutput")

    with tile.TileContext(nc) as tc:
        with (
            tc.tile_pool(name="const", bufs=1) as constp,
            tc.tile_pool(name="zpool", bufs=1) as zp,
            tc.tile_pool(name="stilep", bufs=3) as stp,
            tc.tile_pool(name="otpool", bufs=ND) as otp,
            tc.tile_pool(name="po", bufs=3, space="PSUM") as pop,
            tc.tile_pool(name="psS", bufs=1, space="PSUM") as psp,
        ):
            # mt split so early blocks only wait on their own slice
            MTCH = (2, 6, 8, 8, 8)
            mts = []  # (tile, k0, nblocks)
            k0 = 0
            for qi, qw in enumerate(MTCH):
                t = constp.tile([L, qw * L], bf16, tag=f"mt{qi}", name=f"mt{qi}")
                mts.append((t, k0, qw))
                k0 += qw
            MVH = 4  # mv head blocks
            mvh = constp.tile([L, MVH * MVW], bf16, tag="mvh")
            mvt = constp.tile([L, (NB - MVH) * MVW], bf16, tag="mvt")
            SEH = 8  # selt head blocks
            seh = constp.tile([2 * NB, SEH * L], bf16, tag="seh")
            set_ = constp.tile([2 * NB, (NB - SEH) * L], bf16, tag="set")
            zts = []  # (tile, k0, nblocks)
            zmap = {}  # block k -> (tile, block offset in chunk)
            k0 = 0
            for ci, cw in enumerate(ZCHUNKS):
                zt = zp.tile([L, cw * U], bf16, tag=f"tz{ci}", name=f"tz{ci}")
                zts.append((zt, k0, cw))
                for kk in range(cw):
                    zmap[k0 + kk] = (zt, kk)
                k0 += cw

            # warm-up tile for HAM dummy matmuls (no DMA dependency)
            warm = constp.tile([L, 640], bf16, tag="warm")
            nc.vector.memset(warm[:], 0.0)

            # DMA emission order: first-needed first
            def zdma(i):
                zt, zk0, zw = zts[i]
                nc.sync.dma_start(zt[:], zin[:, zk0 : zk0 + zw, :])

            def mtdma(i):
                t, tk0, tw = mts[i]
                nc.sync.dma_start(t[:], mtT[:, tk0 * L : (tk0 + tw) * L])

            zdma(0)
            mtdma(0)
            nc.sync.dma_start(mvh[:], mvT[:, 0 : MVH * MVW])
            zdma(1)
            nc.sync.dma_start(seh[:], seltT[:, 0 : SEH * L])
            mtdma(1)
            zdma(2)
            zdma(3)
            mtdma(2)
            nc.sync.dma_start(mvt[:], mvT[:, MVH * MVW :])
            zdma(4)
            nc.sync.dma_start(set_[:], seltT[:, SEH * L :])
            zdma(5)
            mtdma(3)
            zdma(6)
            mtdma(4)
            zdma(7)
            zdma(8)
            zdma(9)
            zdma(10)
            zdma(11)

            def rhs_z(k, h):
                zt, kk = zmap[k]
                off = kk * U + h * 512
                return zt[:, off : off + 512]

            def lhs_mt(k):
                for t, tk0, tw in mts:
                    if tk0 <= k < tk0 + tw:
                        return t[:, (k - tk0) * L : (k - tk0 + 1) * L]
                raise AssertionError(k)

            def lhs_mv(k):
                if k < MVH:
                    return mvh[:, k * MVW : (k + 1) * MVW]
                return mvt[:, (k - MVH) * MVW : (k - MVH + 1) * MVW]

            def lhs_selt(k, q):
                p0 = 32 * q
                if k < SEH:
                    return seh[p0 : p0 + NB, k * L : (k + 1) * L]
                return set_[p0 : p0 + NB, (k - SEH) * L : (k - SEH + 1) * L]

            psS = (
                psp.tile([2 * NB, 512], f32, tag="psSa", name="psSa"),
                psp.tile([2 * NB, 512], f32, tag="psSb", name="psSb"),
            )
            stiles = {}
            pos = {}
            ots = {}
            odone = []

            def emit_front(k):
                po_a = pop.tile([L, 512], f32, tag="poa")
                po_b = pop.tile([L, 512], f32, tag="pob")
                po = (po_a, po_b)
                for h in range(2):
                    nc.tensor.matmul(
                        po[h][:, :],
                        lhs_mt(k),
                        rhs_z(k, h),
                        start=True,
                        stop=(k == 0),
                    )
                pos[k] = po
                if k < NB - 1:
                    for h in range(2):
                        nc.tensor.matmul(
                            psS[h][:, :],
                            lhs_mv(k),
                            rhs_z(k, h),
                            start=(k == 0),
                            stop=(k == NB - 2),
                        )
                if k % 2 == 1:
                    c = k // 2
                    st = stp.tile([2 * NB, U], bf16, tag="st")
                    nc.scalar.copy(st[:, 0:512], psS[0][0 : 2 * NB, :])
                    nc.vector.tensor_copy(st[:, 512:U], psS[1][0 : 2 * NB, :])
                    stiles[c] = st

            def emit_drain(k):
                po = pos.pop(k)
                if k % 2 == 0:
                    ot = otp.tile([L, 2 * U], bf16, tag="ot")
                    ots[k // 2] = ot
                ot = ots[k // 2]
                o0 = (k % 2) * U
                nc.vector.tensor_copy(ot[:, o0 : o0 + 512], po[0][:, :])
                nc.scalar.copy(ot[:, o0 + 512 : o0 + U], po[1][:, :])
                if k % 2 == 1:
                    d = k // 2
                    odone.append((d, ots.pop(d)))

            def emit_pack(blocks):
                # paired carries on disjoint 32-row PE groups (concurrent
                # via tile_position); S rows <= max(blocks)-1 are covered
                # by the Scopy of chunk (max-1)//2
                c = (max(blocks) - 1) // 2
                st = stiles[c]
                for h in range(2):
                    for q, k in enumerate(blocks):
                        p0 = 32 * q
                        nc.tensor.matmul(
                            pos[k][h][:, :],
                            lhs_selt(k, q),
                            st[p0 : p0 + NB, h * 512 : h * 512 + 512],
                            start=False,
                            stop=True,
                        )
                for k in blocks:
                    emit_drain(k)

            for w in range(NWARM):
                pw = pop.tile([L, 512], f32, tag="poa" if w % 2 == 0 else "pob")
                nc.tensor.matmul(
                    pw[:, :], warm[:, 0:L], warm[:, L : L + 512],
                    start=True, stop=True,
                )
            for k in range(NB):
                emit_front(k)
                if k == 1:
                    emit_drain(0)
                if k >= 3 and k % 2 == 1:
                    emit_pack((k - 2, k - 1))
            emit_pack((NB - 1,))
            for d, ot_full in odone:
                if d >= ND - 2:
                    nc.sync.dma_start(out[d, :, 0:U], ot_full[:, 0:U])
                    nc.sync.dma_start(out[d, :, U:], ot_full[:, U:])
                else:
                    nc.sync.dma_start(out[d, :, :], ot_full[:])
    nc.finalize()
    return nc


_NC = None


def _get_nc() -> bass.Bass:
    global _NC
    if _NC is None:
        _NC = build_nc()
    return _NC


def prep_in_maps(x: np.ndarray) -> list[dict]:
    maps = []
    sidx = np.arange(L)
    smask = sidx[None, :, None] <= sidx[None, None, :]
    for c in range(B):
        xs = x[c, :, 0].astype(np.float64)
        ys = x[c, :, 1].astype(np.float64)
        z = x[c, :, 2:]
        yb = ys.reshape(NB, L)
        xb = xs.reshape(NB, L)
        cp = np.cumprod(yb, axis=1)  # cp[k,t] = prod_{r=0..t} y_{kL+r}
        ratio = cp[:, None, :] / cp[:, :, None]  # prod_{s+1..t}
        mt = xb[:, :, None] * ratio * smask  # [k, s, t]
        mlast = mt[:, :, L - 1]  # [k, s]
        A = cp[:, L - 1]
        W = np.zeros((NB, NB))
        for r in range(NB):
            W[r, r] = 1.0
            if r:
                W[r, :r] = W[r - 1, :r] * A[r]
        mv2 = mlast[:, :, None] * W.T[:, None, :]  # [k, s, r]
        mv2 = np.concatenate([mv2, mv2], axis=2)  # [k, s, 64] 2 replicas
        selt = np.zeros((NB, NB, L))  # [s, k, t]
        for k in range(1, NB):
            selt[k - 1, k, :] = cp[k]
        selt = np.concatenate([selt, selt], axis=0)  # [64, k, t]

        # zin[p, k, c] = z[k*L + p, c]
        zb = (
            np.ascontiguousarray(z)
            .astype(nbf16)
            .reshape(NB, L, U)
            .transpose(1, 0, 2)
        )
        maps.append(
            {
                "zin": np.ascontiguousarray(zb),
                "mtT": np.ascontiguousarray(
                    mt.transpose(1, 0, 2).reshape(L, NB * L).astype(nbf16)
                ),
                "mvT": np.ascontiguousarray(
                    mv2.transpose(1, 0, 2).reshape(L, NB * 2 * NB).astype(nbf16)
                ),
                "seltT": np.ascontiguousarray(
                    selt.reshape(2 * NB, NB * L).astype(nbf16)
                ),
            }
        )
    return maps


def unpack_out(outb: np.ndarray) -> np.ndarray:
    # outb [B, ND, L, 2U]: out[d, p, j*U+c] = res[(2d+j)L+p, c]
    o = outb.reshape(B, ND, L, 2, U)
    o = o.transpose(0, 1, 3, 2, 4)  # [B, d, j, p, c]
    return np.ascontiguousarray(o).reshape(B, T, U).astype(np.float32)


def kernel(**inputs: np.ndarray) -> np.ndarray:
    x = np.ascontiguousarray(inputs["inputs"], dtype=np.float32)
    assert x.shape == (B, T, F), x.shape
    nc = _get_nc()
    res = run_bass_kernel_spmd(nc, prep_in_maps(x), core_ids=list(range(B)))
    outb = np.stack([res.results[c]["out"] for c in range(B)], axis=0)
    return unpack_out(outb)


# revision 18
# speedup vs baseline: 1.5286x; 1.1137x over previous
"""Trainium2 Bass kernel for CLSProcess: diagonal linear recurrence
state_t = y_t * state_{t-1} + x_t * z_t over [B=8, T=4096, units=1024].

Sharding: batch across the 8 cores (one batch element per core).

v6 design — NO serial inter-block chain on device. The block recurrence
s_k = A_k s_{k-1} + b_k is solved with host-precomputed weights:

  - Host (f64, exact): per-block decay matrices M'_k[s,t] = x_s*prod(y)
    (main-matmul lhsT), the triangular inter-block propagator
    W[r,j] = prod_{i=j+1..r} A_i, "stacked matvec" weights
    mv2_j = outer(mlast_j, W[:,j]) and carry selectors
    selt_k[s,t] = I[s==k-1] * p_{k,t}. All shipped as bf16 sidecars.
  - Device (per core, single pass, N=1024 bf16 matmuls into 2-bank
    PSUM tiles):
      main:   po_k  = M'_k.T @ z_k           (start, no stop)
      matvec: ps_S += mv2_k.T @ z_k          (stacked: row r of ps_S
              accumulates W[r,k]*b_k, so S rows <= k are FINAL after
              matvec k — the inter-block scan happens inside PSUM
              accumulation, progressively, with no barrier)
      every 2 blocks: copy ps_S -> Stile (bf16, triple-buffered; two
              [32,512] copies split across scalar+vector)
      carry:  po_k += selt_k.T @ Stile       (K=32, start=False)
      drain:  one [128,1024] CAST -> bf16 out tile -> DMA out
  - Carries/drains are emitted DELAY=2 blocks behind mains; with CH=2
    the S-copy for row k-1 lands >=2 iterations before carry_k needs
    it, so the in-order PE queue never stalls on the S pipeline.
  - PSUM: 3 po tiles (2 banks each) + ps_S (2 banks) = 8 banks.
  - z is loaded via per-chunk SBUF tiles (sized small-first so block 0
    starts ~1us after the DMA preamble); M' sidecar is split in 4
    separate tiles so main_0 doesn't wait on the full 1MB transfer.
    I/O bf16; output written as [ND, 128, 2048] (2-block 4KB lines).
"""

import numpy as np
import ml_dtypes

import concourse.bacc as bacc
import concourse.bass as bass
import concourse.mybir as mybir
import concourse.tile as tile
from concourse.bass_utils import run_bass_kernel_spmd

B = 8
T = 4096
F = 1026
U = 1024
L = 128
NB = T // L  # 32 blocks
ND = NB // 2  # 16 two-block DMA groups
DELAY = 2  # carry/drain emission lag behind mains (blocks)
# z DMA chunking (in blocks): small chunks first so the pipeline
# starts early, large later to cut DMA instruction count
ZCHUNKS = (1, 1, 1, 2, 2, 2, 3, 4, 4, 4, 4, 4)
NWARM = 6  # HAM warm-up dummy matmuls during the initial load phase
f32 = mybir.dt.float32
bf16 = mybir.dt.bfloat16
nbf16 = ml_dtypes.bfloat16


def build_nc() -> bass.Bass:
    nc = bacc.Bacc()
    # zin[p, k, c] = z_{k*L + p, c}  (per-p 64KB contiguous)
    zin = nc.dram_tensor("zin", [L, NB, U], bf16, kind="ExternalInput")
    # mtT[s, k*L + t] = M'_k[t, s] = x_{kL+s} * prod_{r=kL+s+1..kL+t} y_r
    mtT = nc.dram_tensor("mtT", [L, NB * L], bf16, kind="ExternalInput")
    # mvT[s, k*128 + 32i + r] = W[r, k] * M'_k[L-1, s]  (4 replicas)
    MVW = 4 * NB
    mvT = nc.dram_tensor("mvT", [L, NB * MVW], bf16, kind="ExternalInput")
    # seltT[32i + s, k*L + t] = I[s == k-1] * prod_{r=kL..kL+t} y_r
    seltT = nc.dram_tensor("seltT", [4 * NB, NB * L], bf16, kind="ExternalInput")
    # out[d, p, j*U + c] = out_{(2d+j)*L + p, c}
    out = nc.dram_tensor("out", [ND, L, 2 * U], bf16, kind="ExternalOutput")

    with tile.TileContext(nc) as tc:
        with (
            tc.tile_pool(name="const", bufs=1) as constp,
            tc.tile_pool(name="zpool", bufs=1) as zp,
            tc.tile_pool(name="stilep", bufs=3) as stp,
            tc.tile_pool(name="otpool", bufs=ND) as otp,
            tc.tile_pool(name="po", bufs=3, space="PSUM") as pop,
            tc.tile_pool(name="psS", bufs=1, space="PSUM") as psp,
        ):
            # mt split so early blocks only wait on their own slice
            MTCH = (2, 6, 8, 8, 8)
            mts = []  # (tile, k0, nblocks)
            k0 = 0
            for qi, qw in enumerate(MTCH):
                t = constp.tile([L, qw * L], bf16, tag=f"mt{qi}", name=f"mt{qi}")
                mts.append((t, k0, qw))
                k0 += qw
            MVH = 4  # mv head blocks
            mvh = constp.tile([L, MVH * MVW], bf16, tag="mvh")
            mvt = constp.tile([L, (NB - MVH) * MVW], bf16, tag="mvt")
            SEH = 8  # selt head blocks
            seh = constp.tile([4 * NB, SEH * L], bf16, tag="seh")
            set_ = constp.tile([4 * NB, (NB - SEH) * L], bf16, tag="set")
            zts = []  # (tile, k0, nblocks)
            zmap = {}  # block k -> (tile, block offset in chunk)
            k0 = 0
            for ci, cw in enumerate(ZCHUNKS):
                zt = zp.tile([L, cw * U], bf16, tag=f"tz{ci}", name=f"tz{ci}")
                zts.append((zt, k0, cw))
                for kk in range(cw):
                    zmap[k0 + kk] = (zt, kk)
                k0 += cw

            # warm-up tile for HAM dummy matmuls (no DMA dependency)
            warm = constp.tile([L, 640], bf16, tag="warm")
            nc.vector.memset(warm[:], 0.0)

            # DMA emission order: first-needed first
            def zdma(i):
                zt, zk0, zw = zts[i]
                nc.sync.dma_start(zt[:], zin[:, zk0 : zk0 + zw, :])

            def mtdma(i):
                t, tk0, tw = mts[i]
                nc.sync.dma_start(t[:], mtT[:, tk0 * L : (tk0 + tw) * L])

            zdma(0)
            mtdma(0)
            nc.sync.dma_start(mvh[:], mvT[:, 0 : MVH * MVW])
            zdma(1)
            nc.sync.dma_start(seh[:], seltT[:, 0 : SEH * L])
            mtdma(1)
            zdma(2)
            zdma(3)
            mtdma(2)
            nc.sync.dma_start(mvt[:], mvT[:, MVH * MVW :])
            zdma(4)
            nc.sync.dma_start(set_[:], seltT[:, SEH * L :])
            zdma(5)
            mtdma(3)
            zdma(6)
            mtdma(4)
            zdma(7)
            zdma(8)
            zdma(9)
            zdma(10)
            zdma(11)

            def rhs_z(k, h):
                zt, kk = zmap[k]
                off = kk * U + h * 512
                return zt[:, off : off + 512]

            def lhs_mt(k):
                for t, tk0, tw in mts:
                    if tk0 <= k < tk0 + tw:
                        return t[:, (k - tk0) * L : (k - tk0 + 1) * L]
                raise AssertionError(k)

            def lhs_mv(k):
                if k < MVH:
                    return mvh[:, k * MVW : (k + 1) * MVW]
                return mvt[:, (k - MVH) * MVW : (k - MVH + 1) * MVW]

            def lhs_selt(k, q):
                p0 = 32 * q
                if k < SEH:
                    return seh[p0 : p0 + NB, k * L : (k + 1) * L]
                return set_[p0 : p0 + NB, (k - SEH) * L : (k - SEH + 1) * L]

            psS = (
                psp.tile([4 * NB, 512], f32, tag="psSa", name="psSa"),
                psp.tile([4 * NB, 512], f32, tag="psSb", name="psSb"),
            )
            stiles = {}
            pos = {}
            ots = {}
            odone = []

            def emit_front(k):
                po_a = pop.tile([L, 512], f32, tag="poa")
                po_b = pop.tile([L, 512], f32, tag="pob")
                po = (po_a, po_b)
                for h in range(2):
                    nc.tensor.matmul(
                        po[h][:, :],
                        lhs_mt(k),
                        rhs_z(k, h),
                        start=True,
                        stop=(k == 0),
                    )
                pos[k] = po
                if k < NB - 1:
                    for h in range(2):
                        nc.tensor.matmul(
                            psS[h][:, :],
                            lhs_mv(k),
                            rhs_z(k, h),
                            start=(k == 0),
                            stop=(k == NB - 2),
                        )
                if k % 2 == 1:
                    c = k // 2
                    st = stp.tile([4 * NB, U], bf16, tag="st")
                    nc.scalar.copy(st[:, 0:512], psS[0][0 : 4 * NB, :])
                    nc.vector.tensor_copy(st[:, 512:U], psS[1][0 : 4 * NB, :])
                    stiles[c] = st

            def emit_drain(k):
                po = pos.pop(k)
                if k % 2 == 0:
                    ot = otp.tile([L, 2 * U], bf16, tag="ot")
                    ots[k // 2] = ot
                ot = ots[k // 2]
                o0 = (k % 2) * U
                nc.vector.tensor_copy(ot[:, o0 : o0 + 512], po[0][:, :])
                nc.scalar.copy(ot[:, o0 + 512 : o0 + U], po[1][:, :])
                if k % 2 == 1:
                    d = k // 2
                    odone.append((d, ots.pop(d)))

            def emit_pack(blocks):
                # paired carries on disjoint 32-row PE groups (concurrent
                # via tile_position); S rows <= max(blocks)-1 are covered
                # by the Scopy of chunk (max-1)//2
                c = (max(blocks) - 1) // 2
                st = stiles[c]
                for h in range(2):
                    for q, k in enumerate(blocks):
                        p0 = 32 * q
                        nc.tensor.matmul(
                            pos[k][h][:, :],
                            lhs_selt(k, q),
                            st[p0 : p0 + NB, h * 512 : h * 512 + 512],
                            start=False,
                            stop=True,
                        )
                for k in blocks:
                    emit_drain(k)

            for w in range(NWARM):
                pw = pop.tile([L, 512], f32, tag="poa" if w % 2 == 0 else "pob")
                nc.tensor.matmul(
                    pw[:, :], warm[:, 0:L], warm[:, L : L + 512],
                    start=True, stop=True,
                )
            for k in range(NB):
                emit_front(k)
                if k == 1:
                    emit_drain(0)
                if k >= 3 and k % 2 == 1:
                    emit_pack((k - 2, k - 1))
            emit_pack((NB - 1,))
            for d, ot_full in odone:
                if d >= ND - 2:
                    nc.sync.dma_start(out[d, :, 0:U], ot_full[:, 0:U])
                    nc.sync.dma_start(out[d, :, U:], ot_full[:, U:])
                else:
                    nc.sync.dma_start(out[d, :, :], ot_full[:])
    nc.finalize()
    return nc


_NC = None


def _get_nc() -> bass.Bass:
    global _NC
    if _NC is None:
        _NC = build_nc()
    return _NC


def prep_in_maps(x: np.ndarray) -> list[dict]:
    maps = []
    sidx = np.arange(L)
    smask = sidx[None, :, None] <= sidx[None, None, :]
    for c in range(B):
        xs = x[c, :, 0].astype(np.float64)
        ys = x[c, :, 1].astype(np.float64)
        z = x[c, :, 2:]
        yb = ys.reshape(NB, L)
        xb = xs.reshape(NB, L)
        cp = np.cumprod(yb, axis=1)  # cp[k,t] = prod_{r=0..t} y_{kL+r}
        ratio = cp[:, None, :] / cp[:, :, None]  # prod_{s+1..t}
        mt = xb[:, :, None] * ratio * smask  # [k, s, t]
        mlast = mt[:, :, L - 1]  # [k, s]
        A = cp[:, L - 1]
        W = np.zeros((NB, NB))
        for r in range(NB):
            W[r, r] = 1.0
            if r:
                W[r, :r] = W[r - 1, :r] * A[r]
        mv2 = mlast[:, :, None] * W.T[:, None, :]  # [k, s, r]
        mv2 = np.concatenate([mv2] * 4, axis=2)  # [k, s, 128] 4 replicas
        selt = np.zeros((NB, NB, L))  # [s, k, t]
        for k in range(1, NB):
            selt[k - 1, k, :] = cp[k]
        selt = np.concatenate([selt] * 4, axis=0)  # [128, k, t]

        # zin[p, k, c] = z[k*L + p, c]
        zb = (
            np.ascontiguousarray(z)
            .astype(nbf16)
            .reshape(NB, L, U)
            .transpose(1, 0, 2)
        )
        maps.append(
            {
                "zin": np.ascontiguousarray(zb),
                "mtT": np.ascontiguousarray(
                    mt.transpose(1, 0, 2).reshape(L, NB * L).astype(nbf16)
                ),
                "mvT": np.ascontiguousarray(
                    mv2.transpose(1, 0, 2).reshape(L, NB * 4 * NB).astype(nbf16)
                ),
                "seltT": np.ascontiguousarray(
                    selt.reshape(4 * NB, NB * L).astype(nbf16)
                ),
            }
        )
    return maps


def unpack_out(outb: np.ndarray) -> np.ndarray:
    # outb [B, ND, L, 2U]: out[d, p, j*U+c] = res[(2d+j)L+p, c]
    o = outb.reshape(B, ND, L, 2, U)
    o = o.transpose(0, 1, 3, 2, 4)  # [B, d, j, p, c]
    return np.ascontiguousarray(o).reshape(B, T, U).astype(np.float32)


def kernel(**inputs: np.ndarray) -> np.ndarray:
    x = np.ascontiguousarray(inputs["inputs"], dtype=np.float32)
    assert x.shape == (B, T, F), x.shape
    nc = _get_nc()
    res = run_bass_kernel_spmd(nc, prep_in_maps(x), core_ids=list(range(B)))
    outb = np.stack([res.results[c]["out"] for c in range(B)], axis=0)
    return unpack_out(outb)
